# revision 1
# baseline (speedup 1.0000x reference)
"""CMHSA (1x1-conv multi-head self-attention with a head-mixing 1x1 conv and
instance-norm on the attention maps) as a Trainium2 Bass kernel on 8
NeuronCores.

Reference math (B=4, C=512, T=1024, HEADS=8, hd=64):
  xf = x[b] as [C, T];  q/k/v = W @ xf;  per head h: S_h = q_h^T k_h * hd^-.5
  S'_g = sum_h w_head[g,h] S_h            (head-mixing 1x1 conv)
  A = softmax(S'_g, axis=-1)
  A = instnorm(A) * gamma_g + beta_g      (stats over the whole [T,T] map)
  out = (A @ v_g^T).view(b, t, c) @ w_proj.T + b_proj -> [B, C, H, W]

Transformations:
  * Head-mixing folds into Q: S'_g = (alpha_g . q)^T k with per-channel
    scale alpha_g[o] = w_head[g, o//64].  Each (b, g) map becomes fully
    independent -> 32 maps over 8 cores, 4 maps/core, zero collectives.
  * Logits are ~N(0,1): softmax without max-subtraction is safe.
  * Attention is computed transposed (S^T[T, q]) so the T (softmax) axis is
    the PE contraction axis; softmax row-sums come out of the AV matmul by
    appending 64 ones-columns to the stationary [v_g | 1]: PSUM rows 0-63 =
    v @ E, rows 64-127 = rowsum (pre-broadcast).  A second column-tiled
    matmul with an all-ones stationary reduces E^2 for the variance.
  * gamma/inv_std/beta/b_proj and the constant (beta - a*mu) * sum_T v term
    fold into a host epilogue given per-map sum_q sqsum/rowsum^2, which the
    device emits as a tiny second output.
  * The projection reads the scaled map through a stride-8 access pattern
    that realizes torch's .view(b, t, c) shuffle for free.
"""

import numpy as np

import concourse.bass as bass
import concourse.tile as tile
import concourse.mybir as mybir
from concourse import bacc
from concourse.bass_utils import run_bass_kernel_spmd

F32 = mybir.dt.float32
F32R = mybir.dt.float32r

B, C, HH, WW = 4, 512, 32, 32
T = HH * WW          # 1024
HEADS, HD = 8, 64
EPS = 1e-5
SCALE = HD ** -0.5   # 1/8
NCORES = 8
GPC = HEADS // 2     # 4 maps (g values) per core; 2 cores per batch
CC = C // 128        # 4 contraction chunks
TB = T // 128        # 8 T-blocks
MU = 1.0 / T

_prog_cache = {}


def build_program(reps=1):
    """Build + compile the SPMD Bass program (one NEFF, same for all cores).

    reps>1 repeats the whole compute body (for wall-clock timing via
    wall(reps=R) - wall(reps=1)); input loads run once."""
    if reps in _prog_cache:
        return _prog_cache[reps]

    nc = bacc.Bacc("TRN2", target_bir_lowering=False, debug=False,
                   num_devices=NCORES)

    x_d = nc.dram_tensor("x", [C, T], F32R, kind="ExternalInput")
    wq_d = nc.dram_tensor("wqT", [C, C], F32R, kind="ExternalInput")
    wk_d = nc.dram_tensor("wkT", [C, C], F32R, kind="ExternalInput")
    wv_d = nc.dram_tensor("wvT", [C, GPC * HD], F32R, kind="ExternalInput")
    wp_d = nc.dram_tensor("wpT", [64, 8 * C], F32R, kind="ExternalInput")
    al_d = nc.dram_tensor("alphas", [128, CC * GPC], F32, kind="ExternalInput")
    on_d = nc.dram_tensor("ones", [128, 128], F32R, kind="ExternalInput")
    out_d = nc.dram_tensor("out", [GPC * 128, C], F32, kind="ExternalOutput")
    s2_d = nc.dram_tensor("s2", [GPC, 2], F32, kind="ExternalOutput")

    with tile.TileContext(nc) as tc:
        with (
            tc.tile_pool(name="persist", bufs=1) as persist,
            tc.tile_pool(name="qg", bufs=2) as qg_pool,
            tc.tile_pool(name="e", bufs=3) as e_pool,
            tc.tile_pool(name="e2", bufs=3) as e2_pool,
            tc.tile_pool(name="g", bufs=2) as g_pool,
            tc.tile_pool(name="st", bufs=2) as st_pool,
            tc.tile_pool(name="qkps", bufs=2, space="PSUM") as qk_ps,
            tc.tile_pool(name="avps", bufs=2, space="PSUM") as av_ps,
        ):
            # ---------------- load inputs ----------------
            x_sb = persist.tile([128, CC * T], F32R)   # x[c,t]; chunk cc at cols cc*T
            for cc in range(CC):
                for th in range(2):
                    nc.sync.dma_start(
                        x_sb[:, cc * T + th * 512:cc * T + (th + 1) * 512],
                        x_d[cc * 128:(cc + 1) * 128,
                            th * 512:(th + 1) * 512])
            wq_sb = persist.tile([128, CC * C], F32R)  # w_q.T/8; chunk cc at cols cc*C
            wk_sb = persist.tile([128, CC * C], F32R)
            for w_sb, w_d in ((wq_sb, wq_d), (wk_sb, wk_d)):
                for cc in range(CC):
                    nc.sync.dma_start(w_sb[:, cc * C:(cc + 1) * C],
                                      w_d[cc * 128:(cc + 1) * 128, :])
            wv_sb = persist.tile([128, CC * GPC * HD], F32R)  # this core's v heads
            for cc in range(CC):
                nc.sync.dma_start(
                    wv_sb[:, cc * GPC * HD:(cc + 1) * GPC * HD],
                    wv_d[cc * 128:(cc + 1) * 128, :])
            wp_sb = persist.tile([64, 8 * C], F32R)    # wpT_r[d, jh*512 + c]
            nc.sync.dma_start(wp_sb[:], wp_d[:])
            al_sb = persist.tile([128, CC * GPC], F32)
            nc.sync.dma_start(al_sb[:], al_d[:])
            ones_sb = persist.tile([128, 128], F32R)
            nc.sync.dma_start(ones_sb[:], on_d[:])

            for _rep in range(reps):
                # ---------------- Q, K = W @ x ----------------
                q_sb = persist.tile([128, CC * T], F32R)   # Q[o,t]; chunk ob at cols ob*T
                k_sb = persist.tile([128, CC * T], F32R)
                for w_sb, dst in ((wq_sb, q_sb), (wk_sb, k_sb)):
                    for ob in range(4):
                        ps = qk_ps.tile([128, 1024], F32, tag="mmps", name="qkv_ps")
                        for th in range(2):
                            for cc in range(CC):
                                nc.tensor.matmul(
                                    ps[:, th * 512:(th + 1) * 512],
                                    (w_sb[:, cc * C + ob * 128:
                                            cc * C + (ob + 1) * 128]),
                                    (x_sb[:, cc * T + th * 512:
                                            cc * T + th * 512 + 512]),
                                    start=(cc == 0), stop=(cc == CC - 1))
                        nc.scalar.copy(dst[:, ob * T:(ob + 1) * T], ps[:])

                # ---------------- V' = [v-slices | ones] per T-block ----------------
                # vp block tb (512 cols): [gi*128, gi*128+64) = V^T[:, gi*64..]
                #                         [gi*128+64, gi*128+128) = ones
                vp_sb = persist.tile([128, TB * 512], F32R)
                vp_v = vp_sb[:].rearrange("p (t g k) -> p t g k", g=GPC, k=128)
                for tb in range(TB):
                    nc.sync.dma_start(
                        vp_v[:, tb, :, 64:128],
                        bass.AP(tensor=on_d, offset=0,
                                ap=[[128, 128], [0, GPC], [1, 64]]))
                for tb in range(TB):
                    ps = qk_ps.tile([128, 1024], F32, tag="mmps", name="vt_ps")
                    for cc in range(CC):
                        nc.tensor.matmul(
                            ps[:, 0:GPC * HD],
                            (x_sb[:, cc * T + tb * 128:cc * T + (tb + 1) * 128]),
                            (wv_sb[:, cc * GPC * HD:(cc + 1) * GPC * HD]),
                            start=(cc == 0), stop=(cc == CC - 1))
                    nc.vector.tensor_copy(
                        vp_v[:, tb, :, 0:64],
                        ps[:, 0:GPC * HD].rearrange("p (g k) -> p g k", k=64))

                # ---------------- per-map pipeline ----------------
                # A single emission FIFO keeps every non-QK chunk of work
                # (AV/sqsum matmuls, epilogue, projection) trailing ~2 steps
                # behind the QK stream, across map boundaries, so the PE's
                # static order always has QK matmuls to chew while ACT (exp)
                # and the rowsum-reciprocal chain catch up.

                from collections import deque
                todo = deque()

                def emit_qg(gi):
                    qg_sb = qg_pool.tile([128, CC * T], F32R, tag="qg",
                                         name=f"qg{gi}")
                    for cc in range(CC):
                        nc.vector.tensor_scalar_mul(
                            qg_sb[:, cc * T:(cc + 1) * T],
                            q_sb[:, cc * T:(cc + 1) * T],
                            al_sb[:, cc * GPC + gi:cc * GPC + gi + 1])
                    return qg_sb

                def emit_avsq(gi, tb, avs, sqs, e_t, e2_t):
                    for qh in range(2):
                        nc.tensor.matmul(
                            avs[qh][:, :],
                            vp_sb[:, tb * 512 + gi * 128:
                                  tb * 512 + (gi + 1) * 128],
                            e_t[:, qh * 512:(qh + 1) * 512],
                            start=(tb == 0), stop=(tb == TB - 1))
                        nc.tensor.matmul(
                            sqs[qh][:, :],
                            ones_sb[:],
                            e2_t[:, qh * 512:(qh + 1) * 512],
                            start=(tb == 0), stop=(tb == TB - 1))

                def emit_epilogue(gi, avs, sqs, g_sb):
                    # Copy av/sq PSUM to SBUF immediately (releases the
                    # accumulator banks before the slow reciprocal chain).
                    s2_t = st_pool.tile([128, 2], F32, tag="s2_t",
                                        name="s2_t")
                    avc, sqc = [], []
                    for qh in range(2):
                        a_sb = st_pool.tile([128, 512], F32, tag="avc",
                                            name="a_sb", bufs=3)
                        nc.vector.tensor_copy(a_sb[0:65, :], avs[qh][0:65, :])
                        q_sbuf = st_pool.tile([128, 512], F32, tag="sqc",
                                              name="q_sbuf", bufs=3)
                        nc.vector.tensor_copy(q_sbuf[0:64, :],
                                              sqs[qh][0:64, :])
                        avc.append(a_sb)
                        sqc.append(q_sbuf)
                    for qh in range(2):
                        # r = 1/rowsum via exp(-ln .) on ACT (the pinned
                        # table set has both), broadcast to partitions 0..63
                        # with a 1-row fp32 matmul against a 64-col ones
                        # stationary (contraction dim 1 at partition 64).
                        lnr = st_pool.tile([128, 512], F32, tag="lnr",
                                           name="lnr")
                        nc.scalar.activation(lnr[64:65, :], avc[qh][64:65, :],
                                             mybir.ActivationFunctionType.Ln)
                        rrow = st_pool.tile([128, 512], F32, tag="rrow",
                                            name="rrow")
                        nc.scalar.activation(rrow[64:65, :], lnr[64:65, :],
                                             mybir.ActivationFunctionType.Exp,
                                             scale=-1.0)
                        rb_ps = av_ps.tile([128, 512], F32, tag="sq",
                                           name="rb_ps")
                        nc.tensor.matmul(
                            rb_ps[0:64, :],
                            ones_sb[64:65, 0:64].bitcast(F32),
                            rrow[64:65, :],
                            start=True, stop=True)
                        # G = (v@E) * r
                        nc.vector.tensor_tensor(
                            g_sb[:, qh * 512:(qh + 1) * 512],
                            avc[qh][0:64, :], rb_ps[0:64, :],
                            mybir.AluOpType.mult)
                        # s2[qh] = sum_q sqsum * r^2
                        u = st_pool.tile([128, 512], F32, tag="u", name="u")
                        nc.vector.tensor_tensor(u[0:64, :], sqc[qh][0:64, :],
                                                rb_ps[0:64, :],
                                                mybir.AluOpType.mult)
                        u2 = st_pool.tile([128, 512], F32, tag="u", name="u2")
                        nc.vector.tensor_tensor(u2[0:64, :], u[0:64, :],
                                                rb_ps[0:64, :],
                                                mybir.AluOpType.mult)
                        nc.vector.reduce_sum(s2_t[0:64, qh:qh + 1],
                                             u2[0:64, :],
                                             axis=mybir.AxisListType.X)
                    nc.sync.dma_start(s2_d[gi:gi + 1, :], s2_t[0:1, :])

                def emit_proj(gi, g_sb):
                    # out^T[i,c] = sum_jh sum_d G[d, 8i+jh] * wp[d, jh*512+c]
                    g_v = g_sb[:].rearrange("p (i s) -> p s i", s=8)
                    p_ps = av_ps.tile([128, 512], F32, tag="av", name="p_ps")
                    for jh in range(8):
                        nc.tensor.matmul(p_ps[:], g_v[:, jh, :],
                                         wp_sb[:, jh * C:(jh + 1) * C],
                                         start=(jh == 0), stop=(jh == 7))
                    stage = st_pool.tile([128, 512], F32, tag="stage",
                                         name="stage", bufs=2)
                    nc.scalar.copy(stage[:], p_ps[:])
                    nc.sync.dma_start(out_d[gi * 128:(gi + 1) * 128, :],
                                      stage[:])

                qg_next = emit_qg(0)
                for gi in range(GPC):
                    qg_sb = qg_next
                    avs = tuple(av_ps.tile([128, 512], F32, tag="av",
                                           name=f"av{qh}") for qh in range(2))
                    sqs = tuple(av_ps.tile([128, 512], F32, tag="sq",
                                           name=f"sq{qh}") for qh in range(2))

                    for tb in range(TB):
                        s_ps = qk_ps.tile([128, 1024], F32, tag="mmps",
                                          name="s_ps")
                        for qh in range(2):
                            for oc in range(CC):
                                nc.tensor.matmul(
                                    s_ps[:, qh * 512:(qh + 1) * 512],
                                    k_sb[:, oc * T + tb * 128:
                                         oc * T + (tb + 1) * 128],
                                    qg_sb[:, oc * T + qh * 512:
                                          oc * T + qh * 512 + 512],
                                    start=(oc == 0), stop=(oc == CC - 1))
                        e_t = e_pool.tile([128, 1024], F32R)
                        nc.scalar.activation(e_t[:], s_ps[:],
                                             mybir.ActivationFunctionType.Exp)
                        e2_t = e2_pool.tile([128, 1024], F32R)
                        if tb % 2 == 0:
                            nc.vector.tensor_mul(e2_t[:], e_t[:], e_t[:])
                        else:
                            nc.scalar.activation(
                                e2_t[:], s_ps[:],
                                mybir.ActivationFunctionType.Exp, scale=2.0)
                        todo.append(lambda gi=gi, tb=tb, a=avs, s=sqs,
                                    e=e_t, e2=e2_t:
                                    emit_avsq(gi, tb, a, s, e, e2))
                        if tb == 4 and gi + 1 < GPC:
                            qg_next = emit_qg(gi + 1)
                        while len(todo) > 2:
                            todo.popleft()()
                    g_sb = g_pool.tile([64, 1024], F32R)
                    todo.append(lambda gi=gi, a=avs, s=sqs, g=g_sb:
                                emit_epilogue(gi, a, s, g))
                    todo.append(lambda gi=gi, g=g_sb: emit_proj(gi, g))
                while todo:
                    todo.popleft()()

    _pin_act_table(nc)
    nc.compile()
    _prog_cache[reps] = nc
    return nc


def _pin_act_table(nc):
    """Make Exp/Ln/Copy resolvable only via natural_log_exp_and_others so the
    act-table-load pass keeps one set resident (no per-map Exp<->Ln table
    thrash).  Instance-level override; set ids keep matching act_info.json."""
    import bass_rust as _bass_rust
    from concourse.hw_specs import get_activation_tables

    keep = "natural_log_exp_and_others"
    af = mybir.ActivationFunctionType
    ours = {af.Exp, af.Ln, af.Copy, af.Identity}

    def patched_pass():
        has_activation = any(
            isinstance(i, mybir.InstActivation)
            for b in nc.main_func.blocks for i in b.instructions)
        if not has_activation:
            return
        tables = get_activation_tables(nc.m.arch)
        if keep in tables and ours <= set(tables[keep]):
            tables = {name: (fns if name == keep else set(fns) - ours)
                      for name, fns in tables.items()}
        _bass_rust.insert_act_table_loads(nc, list(tables.items()))

    nc.insert_act_table_loads = patched_pass


def _host_prep(x, w_q, w_k, w_v, w_head, in_gamma, in_beta, w_proj, b_proj):
    """Build the 8 per-core input maps (all fp32 numpy)."""
    x = np.asarray(x, dtype=np.float32)
    w_q = np.asarray(w_q, dtype=np.float32)
    w_k = np.asarray(w_k, dtype=np.float32)
    w_v = np.asarray(w_v, dtype=np.float32)
    w_head = np.asarray(w_head, dtype=np.float32)

    wqT = np.ascontiguousarray(w_q.T) * np.float32(SCALE)
    wkT = np.ascontiguousarray(w_k.T)
    wpT_r = np.ascontiguousarray(
        np.asarray(w_proj, dtype=np.float32)
        .T.reshape(8, 64, C).transpose(1, 0, 2).reshape(64, 8 * C))

    in_maps = []
    p = np.arange(128)
    for core in range(NCORES):
        b = core // 2
        g0 = (core % 2) * GPC
        xc = np.ascontiguousarray(x[b].reshape(C, T))
        wvT = np.ascontiguousarray(w_v.T[:, g0 * HD:(g0 + GPC) * HD])
        al = np.empty((128, CC * GPC), dtype=np.float32)
        for cc in range(CC):
            for gi in range(GPC):
                al[:, cc * GPC + gi] = w_head[g0 + gi, cc * 2 + p // 64]
        in_maps.append({
            "x": xc, "wqT": wqT, "wkT": wkT, "wvT": wvT,
            "wpT": wpT_r, "alphas": al,
            "ones": np.ones((128, 128), dtype=np.float32),
        })
    return in_maps


def _host_finish(results, x, w_v, w_head, in_gamma, in_beta, w_proj, b_proj):
    in_gamma = np.asarray(in_gamma, dtype=np.float32)
    in_beta = np.asarray(in_beta, dtype=np.float32)
    w_proj = np.asarray(w_proj, dtype=np.float32)
    b_proj = np.asarray(b_proj, dtype=np.float32)
    w_v = np.asarray(w_v, dtype=np.float32)
    x = np.asarray(x, dtype=np.float32)

    # collapsed_wp[d, c] = sum_jh w_proj[c, jh*64+d]
    collapsed_wp = w_proj.reshape(C, 8, 64).sum(axis=1).T   # [64, C]
    out = np.empty((B, C, T), dtype=np.float32)
    for core in range(NCORES):
        b = core // 2
        g0 = (core % 2) * GPC
        dev = results[core]["out"]              # [512 i, 512 c]
        s2 = results[core]["s2"].sum(axis=1)    # [GPC]
        sv = w_v @ x[b].reshape(C, T).sum(axis=1)   # [C]
        for gi in range(GPC):
            g = g0 + gi
            var = s2[gi] / float(T * T) - MU * MU
            a = in_gamma[g] / np.sqrt(var + EPS)
            cs = in_beta[g] - a * MU
            bias2 = collapsed_wp.T @ sv[g * HD:(g + 1) * HD]   # [C]
            blk = dev[gi * 128:(gi + 1) * 128, :]              # [128 i, C]
            full = a * blk + (cs * bias2 + b_proj)[None, :]
            out[b, :, g * 128:(g + 1) * 128] = full.T
    return out.reshape(B, C, HH, WW)


def _run(inputs, trace=False, reps=1):
    nc = build_program(reps)
    in_maps = _host_prep(**inputs)
    res = run_bass_kernel_spmd(nc, in_maps, core_ids=list(range(NCORES)),
                               trace=trace)
    out = _host_finish(res.results, inputs["x"], inputs["w_v"],
                       inputs["w_head"], inputs["in_gamma"],
                       inputs["in_beta"], inputs["w_proj"], inputs["b_proj"])
    return out, res


def kernel(**inputs):
    out, _ = _run(inputs, trace=False)
    return out



# revision 2
# speedup vs baseline: 1.5555x; 1.5555x over previous
"""CMHSA (1x1-conv multi-head self-attention with a head-mixing 1x1 conv and
instance-norm on the attention maps) as a Trainium2 Bass kernel on 8
NeuronCores.

Reference math (B=4, C=512, T=1024, HEADS=8, hd=64):
  xf = x[b] as [C, T];  q/k/v = W @ xf;  per head h: S_h = q_h^T k_h * hd^-.5
  S'_g = sum_h w_head[g,h] S_h            (head-mixing 1x1 conv)
  A = softmax(S'_g, axis=-1)
  A = instnorm(A) * gamma_g + beta_g      (stats over the whole [T,T] map)
  out = (A @ v_g^T).view(b, t, c) @ w_proj.T + b_proj -> [B, C, H, W]

Transformations:
  * Head-mixing folds into Q: S'_g = (alpha_g . q)^T k with per-channel
    scale alpha_g[o] = w_head[g, o//64].  Each (b, g) map becomes fully
    independent -> 32 maps over 8 cores, 4 maps/core, zero collectives.
  * Logits are ~N(0,1): softmax without max-subtraction is safe.
  * Attention is computed transposed (S^T[T, q]) so the T (softmax) axis is
    the PE contraction axis; softmax row-sums come out of the AV matmul by
    appending one ones-column to the stationary [v_g | 1]: PSUM rows 0-63 =
    v @ E, row 64 = rowsum.  A second 1-col ones matmul on E^2 gives the
    per-row sqsum for the variance.
  * gamma/inv_std/beta/b_proj and the constant (beta - a*mu) * sum_T v term
    fold into a host epilogue given per-map sum_q sqsum/rowsum^2, which the
    device emits as a tiny second output.
  * The projection reads the scaled map through a stride-8 access pattern
    that realizes torch's .view(b, t, c) shuffle for free.
"""

import numpy as np

import concourse.bass as bass
import concourse.tile as tile
import concourse.mybir as mybir
from concourse import bacc
from concourse.bass_utils import run_bass_kernel_spmd

F32 = mybir.dt.float32

B, C, HH, WW = 4, 512, 32, 32
T = HH * WW          # 1024
HEADS, HD = 8, 64
EPS = 1e-5
SCALE = HD ** -0.5   # 1/8
NCORES = 8
GPC = HEADS // 2     # 4 maps (g values) per core; 2 cores per batch
CC = C // 128        # 4 contraction chunks
TB = T // 128        # 8 T-blocks
MU = 1.0 / T
VG = 65              # per-map stationary width: 64 v-cols + 1 ones-col

_prog_cache = {}


def build_program(reps=1):
    """Build + compile the SPMD Bass program (one NEFF, same for all cores).

    reps>1 repeats the whole compute body (for wall-clock timing via
    wall(reps=R) - wall(reps=1)); input loads run once."""
    if reps in _prog_cache:
        return _prog_cache[reps]

    nc = bacc.Bacc("TRN2", target_bir_lowering=False, debug=False,
                   num_devices=NCORES)

    x_d = nc.dram_tensor("x", [C, T], F32, kind="ExternalInput")
    wq_d = nc.dram_tensor("wqT", [C, C], F32, kind="ExternalInput")
    wk_d = nc.dram_tensor("wkT", [C, C], F32, kind="ExternalInput")
    wv_d = nc.dram_tensor("wvT", [C, GPC * HD], F32, kind="ExternalInput")
    wp_d = nc.dram_tensor("wpT", [64, 8 * C], F32, kind="ExternalInput")
    al_d = nc.dram_tensor("alphas", [128, CC * GPC], F32, kind="ExternalInput")
    on_d = nc.dram_tensor("ones", [128, 128], F32, kind="ExternalInput")
    out_d = nc.dram_tensor("out", [GPC * 128, C], F32, kind="ExternalOutput")
    s2_d = nc.dram_tensor("s2", [GPC, 2], F32, kind="ExternalOutput")

    with tile.TileContext(nc) as tc:
        with (
            tc.tile_pool(name="persist", bufs=1) as persist,
            tc.tile_pool(name="qg", bufs=2) as qg_pool,
            tc.tile_pool(name="e", bufs=3) as e_pool,
            tc.tile_pool(name="e2", bufs=3) as e2_pool,
            tc.tile_pool(name="g", bufs=2) as g_pool,
            tc.tile_pool(name="st", bufs=2) as st_pool,
            tc.tile_pool(name="qkps", bufs=2, space="PSUM") as qk_ps,
            tc.tile_pool(name="avps", bufs=2, space="PSUM") as av_ps,
        ):
            # ---------------- load inputs ----------------
            x_sb = persist.tile([128, CC * T], F32)   # x[c,t]; chunk cc at cols cc*T
            for cc in range(CC):
                for th in range(2):
                    nc.sync.dma_start(
                        x_sb[:, cc * T + th * 512:cc * T + (th + 1) * 512],
                        x_d[cc * 128:(cc + 1) * 128,
                            th * 512:(th + 1) * 512])
            wq_sb = persist.tile([128, CC * C], F32)  # w_q.T/8; chunk cc at cols cc*C
            wk_sb = persist.tile([128, CC * C], F32)
            for w_sb, w_d in ((wq_sb, wq_d), (wk_sb, wk_d)):
                for cc in range(CC):
                    nc.sync.dma_start(w_sb[:, cc * C:(cc + 1) * C],
                                      w_d[cc * 128:(cc + 1) * 128, :])
            wv_sb = persist.tile([128, CC * GPC * HD], F32)  # this core's v heads
            for cc in range(CC):
                nc.sync.dma_start(
                    wv_sb[:, cc * GPC * HD:(cc + 1) * GPC * HD],
                    wv_d[cc * 128:(cc + 1) * 128, :])
            wp_sb = persist.tile([64, 8 * C], F32)    # wpT_r[d, jh*512 + c]
            nc.sync.dma_start(wp_sb[:], wp_d[:])
            al_sb = persist.tile([128, CC * GPC], F32)
            nc.sync.dma_start(al_sb[:], al_d[:])
            ones_sb = persist.tile([128, 128], F32)
            nc.sync.dma_start(ones_sb[:], on_d[:])

            # V' layout: [128, TB * GPC * VG]; group (tb, gi) holds the 64
            # v-columns of map gi for T-block tb plus one ones-column.  The
            # ones-columns are written once, outside the rep loop.
            vp_sb = persist.tile([128, TB * GPC * VG], F32)
            vp_v = vp_sb[:].rearrange("p (t g k) -> p t g k", g=GPC, k=VG)
            for tb in range(TB):
                for gi in range(GPC):
                    nc.vector.tensor_copy(vp_v[:, tb, gi, 64:65],
                                          ones_sb[:, 0:1])

            for _rep in range(reps):
                # ---------------- Q, K = W @ x ----------------
                q_sb = persist.tile([128, CC * T], F32)   # Q[o,t]; chunk ob at cols ob*T
                k_sb = persist.tile([128, CC * T], F32)
                for w_sb, dst in ((wq_sb, q_sb), (wk_sb, k_sb)):
                    for ob in range(4):
                        ps = qk_ps.tile([128, 1024], F32, tag="mmps", name="qkv_ps")
                        for th in range(2):
                            for cc in range(CC):
                                nc.tensor.matmul(
                                    ps[:, th * 512:(th + 1) * 512],
                                    (w_sb[:, cc * C + ob * 128:
                                            cc * C + (ob + 1) * 128]),
                                    (x_sb[:, cc * T + th * 512:
                                            cc * T + th * 512 + 512]),
                                    start=(cc == 0), stop=(cc == CC - 1))
                        nc.scalar.copy(dst[:, ob * T:(ob + 1) * T], ps[:])

                # ---------------- V' v-columns per T-block ----------------
                for tb in range(TB):
                    ps = qk_ps.tile([128, 1024], F32, tag="mmps", name="vt_ps")
                    for cc in range(CC):
                        nc.tensor.matmul(
                            ps[:, 0:GPC * HD],
                            (x_sb[:, cc * T + tb * 128:cc * T + (tb + 1) * 128]),
                            (wv_sb[:, cc * GPC * HD:(cc + 1) * GPC * HD]),
                            start=(cc == 0), stop=(cc == CC - 1))
                    nc.vector.tensor_copy(
                        vp_v[:, tb, :, 0:64],
                        ps[:, 0:GPC * HD].rearrange("p (g k) -> p g k", k=64))

                # ---------------- per-map pipeline ----------------
                def emit_qg(gi):
                    qg_sb = qg_pool.tile([128, CC * T], F32, tag="qg",
                                         name=f"qg{gi}")
                    for cc in range(CC):
                        nc.vector.tensor_scalar_mul(
                            qg_sb[:, cc * T:(cc + 1) * T],
                            q_sb[:, cc * T:(cc + 1) * T],
                            al_sb[:, cc * GPC + gi:cc * GPC + gi + 1])
                    return qg_sb

                for gi in range(GPC):
                    qg_sb = emit_qg(gi)
                    avs = tuple(av_ps.tile([128, 512], F32, tag="av",
                                           name=f"av{qh}") for qh in range(2))
                    sqs = tuple(av_ps.tile([128, 512], F32, tag="sq",
                                           name=f"sq{qh}") for qh in range(2))

                    for tb in range(TB):
                        s_ps = qk_ps.tile([128, 1024], F32, tag="mmps",
                                          name="s_ps")
                        for qh in range(2):
                            for oc in range(CC):
                                nc.tensor.matmul(
                                    s_ps[:, qh * 512:(qh + 1) * 512],
                                    k_sb[:, oc * T + tb * 128:
                                         oc * T + (tb + 1) * 128],
                                    qg_sb[:, oc * T + qh * 512:
                                          oc * T + qh * 512 + 512],
                                    start=(oc == 0), stop=(oc == CC - 1))
                        e_t = e_pool.tile([128, 1024], F32)
                        nc.scalar.activation(e_t[:], s_ps[:],
                                             mybir.ActivationFunctionType.Exp)
                        e2_t = e2_pool.tile([128, 1024], F32)
                        nc.vector.tensor_mul(e2_t[:], e_t[:], e_t[:])
                        for qh in range(2):
                            # rows 0-63: v @ E; row 64: rowsum of E
                            nc.tensor.matmul(
                                avs[qh][0:VG, :],
                                vp_v[:, tb, gi, 0:VG],
                                e_t[:, qh * 512:(qh + 1) * 512],
                                start=(tb == 0), stop=(tb == TB - 1))
                            # row 0: per-q sqsum of E
                            nc.tensor.matmul(
                                sqs[qh][0:1, :],
                                ones_sb[:, 0:1],
                                e2_t[:, qh * 512:(qh + 1) * 512],
                                start=(tb == 0), stop=(tb == TB - 1))

                    # ---------------- epilogue ----------------
                    g_sb = g_pool.tile([64, 1024], F32)
                    s2_t = st_pool.tile([128, 2], F32, tag="s2_t", name="s2_t")
                    avc, sqc = [], []
                    for qh in range(2):
                        a_sb = st_pool.tile([128, 512], F32, tag="avc",
                                            name="a_sb", bufs=3)
                        nc.vector.tensor_copy(a_sb[0:VG, :], avs[qh][0:VG, :])
                        q_sbuf = st_pool.tile([128, 512], F32, tag="sqc",
                                              name="q_sbuf", bufs=3)
                        nc.vector.tensor_copy(q_sbuf[0:1, :], sqs[qh][0:1, :])
                        avc.append(a_sb)
                        sqc.append(q_sbuf)
                    for qh in range(2):
                        # r = 1/rowsum via exp(-ln .) on ACT (the pinned
                        # table set has both), broadcast to partitions 0..63
                        # with a 1-row fp32 matmul against a 64-col ones
                        # stationary (contraction dim 1 at partition 64).
                        lnr = st_pool.tile([128, 512], F32, tag="lnr",
                                           name="lnr")
                        nc.scalar.activation(lnr[64:65, :], avc[qh][64:65, :],
                                             mybir.ActivationFunctionType.Ln)
                        rrow = st_pool.tile([128, 512], F32, tag="rrow",
                                            name="rrow")
                        nc.scalar.activation(rrow[64:65, :], lnr[64:65, :],
                                             mybir.ActivationFunctionType.Exp,
                                             scale=-1.0)
                        rb_ps = av_ps.tile([128, 512], F32, tag="sq",
                                           name="rb_ps")
                        nc.tensor.matmul(
                            rb_ps[0:64, :],
                            ones_sb[64:65, 0:64],
                            rrow[64:65, :],
                            start=True, stop=True)
                        # G = (v@E) * r
                        nc.vector.tensor_tensor(
                            g_sb[:, qh * 512:(qh + 1) * 512],
                            avc[qh][0:64, :], rb_ps[0:64, :],
                            mybir.AluOpType.mult)
                        # s2[qh] = sum_q sqsum * r^2
                        u = st_pool.tile([128, 512], F32, tag="u", name="u")
                        nc.vector.tensor_tensor(u[0:1, :], sqc[qh][0:1, :],
                                                rb_ps[0:1, :],
                                                mybir.AluOpType.mult)
                        u2 = st_pool.tile([128, 512], F32, tag="u", name="u2")
                        nc.vector.tensor_tensor(u2[0:1, :], u[0:1, :],
                                                rb_ps[0:1, :],
                                                mybir.AluOpType.mult)
                        nc.vector.reduce_sum(s2_t[0:1, qh:qh + 1],
                                             u2[0:1, :],
                                             axis=mybir.AxisListType.X)
                    nc.sync.dma_start(s2_d[gi:gi + 1, :], s2_t[0:1, :])

                    # ---------------- projection ----------------
                    # out^T[i,c] = sum_jh sum_d G[d, 8i+jh] * wp[d, jh*512+c]
                    g_v = g_sb[:].rearrange("p (i s) -> p s i", s=8)
                    p_ps = av_ps.tile([128, 512], F32, tag="av", name="p_ps")
                    for jh in range(8):
                        nc.tensor.matmul(p_ps[:], g_v[:, jh, :],
                                         wp_sb[:, jh * C:(jh + 1) * C],
                                         start=(jh == 0), stop=(jh == 7))
                    stage = st_pool.tile([128, 512], F32, tag="stage",
                                         name="stage", bufs=2)
                    nc.scalar.copy(stage[:], p_ps[:])
                    nc.sync.dma_start(out_d[gi * 128:(gi + 1) * 128, :],
                                      stage[:])

    _pin_act_table(nc)
    nc.compile()
    _prog_cache[reps] = nc
    return nc


def _pin_act_table(nc):
    """Make Exp/Ln/Copy resolvable only via natural_log_exp_and_others so the
    act-table-load pass keeps one set resident (no per-map Exp<->Ln table
    thrash).  Instance-level override; set ids keep matching act_info.json."""
    import bass_rust as _bass_rust
    from concourse.hw_specs import get_activation_tables

    keep = "natural_log_exp_and_others"
    af = mybir.ActivationFunctionType
    ours = {af.Exp, af.Ln, af.Copy, af.Identity}

    def patched_pass():
        has_activation = any(
            isinstance(i, mybir.InstActivation)
            for b in nc.main_func.blocks for i in b.instructions)
        if not has_activation:
            return
        tables = get_activation_tables(nc.m.arch)
        if keep in tables and ours <= set(tables[keep]):
            tables = {name: (fns if name == keep else set(fns) - ours)
                      for name, fns in tables.items()}
        _bass_rust.insert_act_table_loads(nc, list(tables.items()))

    nc.insert_act_table_loads = patched_pass


def _host_prep(x, w_q, w_k, w_v, w_head, in_gamma, in_beta, w_proj, b_proj):
    """Build the 8 per-core input maps (all fp32 numpy)."""
    x = np.asarray(x, dtype=np.float32)
    w_q = np.asarray(w_q, dtype=np.float32)
    w_k = np.asarray(w_k, dtype=np.float32)
    w_v = np.asarray(w_v, dtype=np.float32)
    w_head = np.asarray(w_head, dtype=np.float32)

    wqT = np.ascontiguousarray(w_q.T) * np.float32(SCALE)
    wkT = np.ascontiguousarray(w_k.T)
    wpT_r = np.ascontiguousarray(
        np.asarray(w_proj, dtype=np.float32)
        .T.reshape(8, 64, C).transpose(1, 0, 2).reshape(64, 8 * C))

    in_maps = []
    p = np.arange(128)
    for core in range(NCORES):
        b = core // 2
        g0 = (core % 2) * GPC
        xc = np.ascontiguousarray(x[b].reshape(C, T))
        wvT = np.ascontiguousarray(w_v.T[:, g0 * HD:(g0 + GPC) * HD])
        al = np.empty((128, CC * GPC), dtype=np.float32)
        for cc in range(CC):
            for gi in range(GPC):
                al[:, cc * GPC + gi] = w_head[g0 + gi, cc * 2 + p // 64]
        in_maps.append({
            "x": xc, "wqT": wqT, "wkT": wkT, "wvT": wvT,
            "wpT": wpT_r, "alphas": al,
            "ones": np.ones((128, 128), dtype=np.float32),
        })
    return in_maps


def _host_finish(results, x, w_v, w_head, in_gamma, in_beta, w_proj, b_proj):
    in_gamma = np.asarray(in_gamma, dtype=np.float32)
    in_beta = np.asarray(in_beta, dtype=np.float32)
    w_proj = np.asarray(w_proj, dtype=np.float32)
    b_proj = np.asarray(b_proj, dtype=np.float32)
    w_v = np.asarray(w_v, dtype=np.float32)
    x = np.asarray(x, dtype=np.float32)

    # collapsed_wp[d, c] = sum_jh w_proj[c, jh*64+d]
    collapsed_wp = w_proj.reshape(C, 8, 64).sum(axis=1).T   # [64, C]
    out = np.empty((B, C, T), dtype=np.float32)
    for core in range(NCORES):
        b = core // 2
        g0 = (core % 2) * GPC
        dev = results[core]["out"]              # [512 i, 512 c]
        s2 = results[core]["s2"].sum(axis=1)    # [GPC]
        sv = w_v @ x[b].reshape(C, T).sum(axis=1)   # [C]
        for gi in range(GPC):
            g = g0 + gi
            var = s2[gi] / float(T * T) - MU * MU
            a = in_gamma[g] / np.sqrt(var + EPS)
            cs = in_beta[g] - a * MU
            bias2 = collapsed_wp.T @ sv[g * HD:(g + 1) * HD]   # [C]
            blk = dev[gi * 128:(gi + 1) * 128, :]              # [128 i, C]
            full = a * blk + (cs * bias2 + b_proj)[None, :]
            out[b, :, g * 128:(g + 1) * 128] = full.T
    return out.reshape(B, C, HH, WW)


def _run(inputs, trace=False, reps=1):
    nc = build_program(reps)
    in_maps = _host_prep(**inputs)
    res = run_bass_kernel_spmd(nc, in_maps, core_ids=list(range(NCORES)),
                               trace=trace)
    out = _host_finish(res.results, inputs["x"], inputs["w_v"],
                       inputs["w_head"], inputs["in_gamma"],
                       inputs["in_beta"], inputs["w_proj"], inputs["b_proj"])
    return out, res


def kernel(**inputs):
    out, _ = _run(inputs, trace=False)
    return out


# revision 4
# speedup vs baseline: 70.2067x; 45.1347x over previous
"""CMHSA (1x1-conv multi-head self-attention with a head-mixing 1x1 conv and
instance-norm on the attention maps) as a Trainium2 Bass kernel on 8
NeuronCores.

Reference math (B=4, C=512, T=1024, HEADS=8, hd=64):
  xf = x[b] as [C, T];  q/k/v = W @ xf;  per head h: S_h = q_h^T k_h * hd^-.5
  S'_g = sum_h w_head[g,h] S_h            (head-mixing 1x1 conv)
  A = softmax(S'_g, axis=-1)
  A = instnorm(A) * gamma_g + beta_g      (stats over the whole [T,T] map)
  out = (A @ v_g^T).view(b, t, c) @ w_proj.T + b_proj -> [B, C, H, W]

Transformations:
  * Head-mixing folds into Q: S'_g = (alpha_g . q)^T k with per-channel
    scale alpha_g[o] = w_head[g, o//64].  Each (b, g) map becomes fully
    independent -> 32 maps over 8 cores, 4 maps/core, zero collectives.
  * Logits are ~N(0,1): softmax without max-subtraction is safe.
  * Attention is computed transposed (S^T[T, q]) so the T (softmax) axis is
    the PE contraction axis; softmax row-sums come out of the AV matmul by
    appending one ones-column to the stationary [v_g | 1]: PSUM rows 0-63 =
    v @ E, row 64 = rowsum.  A second 1-col ones matmul on E^2 gives the
    per-row sqsum for the variance.
  * gamma/inv_std/beta/b_proj and the constant (beta - a*mu) * sum_T v term
    fold into a host epilogue given per-map sum_q sqsum/rowsum^2, which the
    device emits as a tiny second output.
  * The projection reads the scaled map through a stride-8 access pattern
    that realizes torch's .view(b, t, c) shuffle for free.
"""

import os
import tempfile

import numpy as np

import jax

# Persistent XLA/NEFF compilation cache: without it every launch re-runs the
# client-side BIR->NEFF compile (~0.1s/1000 instructions), which dominates
# repeat-call latency.  Harmless if the dir is not writable.
try:
    _cache_dir = os.environ.get(
        "KERNEL_JAX_CACHE", os.path.join(tempfile.gettempdir(), "jaxcache"))
    os.makedirs(_cache_dir, exist_ok=True)
    jax.config.update("jax_compilation_cache_dir", _cache_dir)
    jax.config.update("jax_persistent_cache_min_compile_time_secs", 0.0)
    jax.config.update("jax_persistent_cache_min_entry_size_bytes", 0)
except Exception:
    pass

import concourse.bass as bass
import concourse.tile as tile
import concourse.mybir as mybir
from concourse import bacc
from concourse.bass_utils import run_bass_kernel_spmd

F32 = mybir.dt.float32

B, C, HH, WW = 4, 512, 32, 32
T = HH * WW          # 1024
HEADS, HD = 8, 64
EPS = 1e-5
SCALE = HD ** -0.5   # 1/8
NCORES = 8
GPC = HEADS // 2     # 4 maps (g values) per core; 2 cores per batch
CC = C // 128        # 4 contraction chunks
TB = T // 128        # 8 T-blocks
MU = 1.0 / T
VG = 65              # per-map stationary width: 64 v-cols + 1 ones-col

_prog_cache = {}


def build_program(reps=1):
    """Build + compile the SPMD Bass program (one NEFF, same for all cores).

    reps>1 repeats the whole compute body (for wall-clock timing via
    wall(reps=R) - wall(reps=1)); input loads run once."""
    if reps in _prog_cache:
        return _prog_cache[reps]

    nc = bacc.Bacc("TRN2", target_bir_lowering=False, debug=False,
                   num_devices=NCORES)

    x_d = nc.dram_tensor("x", [C, T], F32, kind="ExternalInput")
    wq_d = nc.dram_tensor("wqT", [C, C], F32, kind="ExternalInput")
    wk_d = nc.dram_tensor("wkT", [C, C], F32, kind="ExternalInput")
    wv_d = nc.dram_tensor("wvT", [C, GPC * HD], F32, kind="ExternalInput")
    wp_d = nc.dram_tensor("wpT", [64, 8 * C], F32, kind="ExternalInput")
    al_d = nc.dram_tensor("alphas", [128, CC * GPC], F32, kind="ExternalInput")
    on_d = nc.dram_tensor("ones", [128, 128], F32, kind="ExternalInput")
    out_d = nc.dram_tensor("out", [GPC * 128, C], F32, kind="ExternalOutput")
    s2_d = nc.dram_tensor("s2", [GPC, 2], F32, kind="ExternalOutput")

    with tile.TileContext(nc) as tc:
        with (
            tc.tile_pool(name="persist", bufs=1) as persist,
            tc.tile_pool(name="qg", bufs=2) as qg_pool,
            tc.tile_pool(name="e", bufs=3) as e_pool,
            tc.tile_pool(name="e2", bufs=3) as e2_pool,
            tc.tile_pool(name="g", bufs=2) as g_pool,
            tc.tile_pool(name="st", bufs=2) as st_pool,
            tc.tile_pool(name="qkps", bufs=2, space="PSUM") as qk_ps,
            tc.tile_pool(name="avps", bufs=2, space="PSUM") as av_ps,
        ):
            # ---------------- load inputs ----------------
            x_sb = persist.tile([128, CC * T], F32)   # x[c,t]; chunk cc at cols cc*T
            for cc in range(CC):
                for th in range(2):
                    nc.sync.dma_start(
                        x_sb[:, cc * T + th * 512:cc * T + (th + 1) * 512],
                        x_d[cc * 128:(cc + 1) * 128,
                            th * 512:(th + 1) * 512])
            wq_sb = persist.tile([128, CC * C], F32)  # w_q.T/8; chunk cc at cols cc*C
            wk_sb = persist.tile([128, CC * C], F32)
            for w_sb, w_d in ((wq_sb, wq_d), (wk_sb, wk_d)):
                for cc in range(CC):
                    nc.sync.dma_start(w_sb[:, cc * C:(cc + 1) * C],
                                      w_d[cc * 128:(cc + 1) * 128, :])
            wv_sb = persist.tile([128, CC * GPC * HD], F32)  # this core's v heads
            for cc in range(CC):
                nc.sync.dma_start(
                    wv_sb[:, cc * GPC * HD:(cc + 1) * GPC * HD],
                    wv_d[cc * 128:(cc + 1) * 128, :])
            wp_sb = persist.tile([64, 8 * C], F32)    # wpT_r[d, jh*512 + c]
            nc.sync.dma_start(wp_sb[:], wp_d[:])
            al_sb = persist.tile([128, CC * GPC], F32)
            nc.sync.dma_start(al_sb[:], al_d[:])
            ones_sb = persist.tile([128, 128], F32)
            nc.sync.dma_start(ones_sb[:], on_d[:])

            # V' layout: [128, TB * GPC * VG]; group (tb, gi) holds the 64
            # v-columns of map gi for T-block tb plus one ones-column.  The
            # ones-columns are written once, outside the rep loop.
            vp_sb = persist.tile([128, TB * GPC * VG], F32)
            vp_v = vp_sb[:].rearrange("p (t g k) -> p t g k", g=GPC, k=VG)
            for tb in range(TB):
                for gi in range(GPC):
                    nc.vector.tensor_copy(vp_v[:, tb, gi, 64:65],
                                          ones_sb[:, 0:1])

            for _rep in range(reps):
                # ---------------- Q, K = W @ x ----------------
                q_sb = persist.tile([128, CC * T], F32)   # Q[o,t]; chunk ob at cols ob*T
                k_sb = persist.tile([128, CC * T], F32)
                for w_sb, dst in ((wq_sb, q_sb), (wk_sb, k_sb)):
                    for ob in range(4):
                        ps = qk_ps.tile([128, 1024], F32, tag="mmps", name="qkv_ps")
                        for th in range(2):
                            for cc in range(CC):
                                nc.tensor.matmul(
                                    ps[:, th * 512:(th + 1) * 512],
                                    (w_sb[:, cc * C + ob * 128:
                                            cc * C + (ob + 1) * 128]),
                                    (x_sb[:, cc * T + th * 512:
                                            cc * T + th * 512 + 512]),
                                    start=(cc == 0), stop=(cc == CC - 1))
                        nc.scalar.copy(dst[:, ob * T:(ob + 1) * T], ps[:])

                # ---------------- V' v-columns per T-block ----------------
                for tb in range(TB):
                    ps = qk_ps.tile([128, 1024], F32, tag="mmps", name="vt_ps")
                    for cc in range(CC):
                        nc.tensor.matmul(
                            ps[:, 0:GPC * HD],
                            (x_sb[:, cc * T + tb * 128:cc * T + (tb + 1) * 128]),
                            (wv_sb[:, cc * GPC * HD:(cc + 1) * GPC * HD]),
                            start=(cc == 0), stop=(cc == CC - 1))
                    nc.vector.tensor_copy(
                        vp_v[:, tb, :, 0:64],
                        ps[:, 0:GPC * HD].rearrange("p (g k) -> p g k", k=64))

                # ---------------- per-map pipeline ----------------
                def emit_qg(gi):
                    qg_sb = qg_pool.tile([128, CC * T], F32, tag="qg",
                                         name=f"qg{gi}")
                    for cc in range(CC):
                        nc.vector.tensor_scalar_mul(
                            qg_sb[:, cc * T:(cc + 1) * T],
                            q_sb[:, cc * T:(cc + 1) * T],
                            al_sb[:, cc * GPC + gi:cc * GPC + gi + 1])
                    return qg_sb

                for gi in range(GPC):
                    qg_sb = emit_qg(gi)
                    avs = tuple(av_ps.tile([128, 512], F32, tag="av",
                                           name=f"av{qh}") for qh in range(2))
                    sqs = tuple(av_ps.tile([128, 512], F32, tag="sq",
                                           name=f"sq{qh}") for qh in range(2))

                    for tb in range(TB):
                        s_ps = qk_ps.tile([128, 1024], F32, tag="mmps",
                                          name="s_ps")
                        for qh in range(2):
                            for oc in range(CC):
                                nc.tensor.matmul(
                                    s_ps[:, qh * 512:(qh + 1) * 512],
                                    k_sb[:, oc * T + tb * 128:
                                         oc * T + (tb + 1) * 128],
                                    qg_sb[:, oc * T + qh * 512:
                                          oc * T + qh * 512 + 512],
                                    start=(oc == 0), stop=(oc == CC - 1))
                        e_t = e_pool.tile([128, 1024], F32)
                        nc.scalar.activation(e_t[:], s_ps[:],
                                             mybir.ActivationFunctionType.Exp)
                        e2_t = e2_pool.tile([128, 1024], F32)
                        nc.vector.tensor_mul(e2_t[:], e_t[:], e_t[:])
                        for qh in range(2):
                            # rows 0-63: v @ E; row 64: rowsum of E
                            nc.tensor.matmul(
                                avs[qh][0:VG, :],
                                vp_v[:, tb, gi, 0:VG],
                                e_t[:, qh * 512:(qh + 1) * 512],
                                start=(tb == 0), stop=(tb == TB - 1))
                            # row 0: per-q sqsum of E
                            nc.tensor.matmul(
                                sqs[qh][0:1, :],
                                ones_sb[:, 0:1],
                                e2_t[:, qh * 512:(qh + 1) * 512],
                                start=(tb == 0), stop=(tb == TB - 1))

                    # ---------------- epilogue ----------------
                    g_sb = g_pool.tile([64, 1024], F32)
                    s2_t = st_pool.tile([128, 2], F32, tag="s2_t", name="s2_t")
                    avc, sqc = [], []
                    for qh in range(2):
                        a_sb = st_pool.tile([128, 512], F32, tag="avc",
                                            name="a_sb", bufs=3)
                        nc.vector.tensor_copy(a_sb[0:VG, :], avs[qh][0:VG, :])
                        q_sbuf = st_pool.tile([128, 512], F32, tag="sqc",
                                              name="q_sbuf", bufs=3)
                        nc.vector.tensor_copy(q_sbuf[0:1, :], sqs[qh][0:1, :])
                        avc.append(a_sb)
                        sqc.append(q_sbuf)
                    for qh in range(2):
                        # r = 1/rowsum via exp(-ln .) on ACT (the pinned
                        # table set has both), broadcast to partitions 0..63
                        # with a 1-row fp32 matmul against a 64-col ones
                        # stationary (contraction dim 1 at partition 64).
                        lnr = st_pool.tile([128, 512], F32, tag="lnr",
                                           name="lnr")
                        nc.scalar.activation(lnr[64:65, :], avc[qh][64:65, :],
                                             mybir.ActivationFunctionType.Ln)
                        rrow = st_pool.tile([128, 512], F32, tag="rrow",
                                            name="rrow")
                        nc.scalar.activation(rrow[64:65, :], lnr[64:65, :],
                                             mybir.ActivationFunctionType.Exp,
                                             scale=-1.0)
                        rb_ps = av_ps.tile([128, 512], F32, tag="sq",
                                           name="rb_ps")
                        nc.tensor.matmul(
                            rb_ps[0:64, :],
                            ones_sb[64:65, 0:64],
                            rrow[64:65, :],
                            start=True, stop=True)
                        # G = (v@E) * r
                        nc.vector.tensor_tensor(
                            g_sb[:, qh * 512:(qh + 1) * 512],
                            avc[qh][0:64, :], rb_ps[0:64, :],
                            mybir.AluOpType.mult)
                        # s2[qh] = sum_q sqsum * r^2
                        u = st_pool.tile([128, 512], F32, tag="u", name="u")
                        nc.vector.tensor_tensor(u[0:1, :], sqc[qh][0:1, :],
                                                rb_ps[0:1, :],
                                                mybir.AluOpType.mult)
                        u2 = st_pool.tile([128, 512], F32, tag="u", name="u2")
                        nc.vector.tensor_tensor(u2[0:1, :], u[0:1, :],
                                                rb_ps[0:1, :],
                                                mybir.AluOpType.mult)
                        nc.vector.reduce_sum(s2_t[0:1, qh:qh + 1],
                                             u2[0:1, :],
                                             axis=mybir.AxisListType.X)
                    nc.sync.dma_start(s2_d[gi:gi + 1, :], s2_t[0:1, :])

                    # ---------------- projection ----------------
                    # out^T[i,c] = sum_jh sum_d G[d, 8i+jh] * wp[d, jh*512+c]
                    g_v = g_sb[:].rearrange("p (i s) -> p s i", s=8)
                    p_ps = av_ps.tile([128, 512], F32, tag="av", name="p_ps")
                    for jh in range(8):
                        nc.tensor.matmul(p_ps[:], g_v[:, jh, :],
                                         wp_sb[:, jh * C:(jh + 1) * C],
                                         start=(jh == 0), stop=(jh == 7))
                    stage = st_pool.tile([128, 512], F32, tag="stage",
                                         name="stage", bufs=2)
                    nc.scalar.copy(stage[:], p_ps[:])
                    nc.sync.dma_start(out_d[gi * 128:(gi + 1) * 128, :],
                                      stage[:])

    _pin_act_table(nc)
    nc.compile()
    _prog_cache[reps] = nc
    return nc


def _pin_act_table(nc):
    """Make Exp/Ln/Copy resolvable only via natural_log_exp_and_others so the
    act-table-load pass keeps one set resident (no per-map Exp<->Ln table
    thrash).  Instance-level override; set ids keep matching act_info.json."""
    import bass_rust as _bass_rust
    from concourse.hw_specs import get_activation_tables

    keep = "natural_log_exp_and_others"
    af = mybir.ActivationFunctionType
    ours = {af.Exp, af.Ln, af.Copy, af.Identity}

    def patched_pass():
        has_activation = any(
            isinstance(i, mybir.InstActivation)
            for b in nc.main_func.blocks for i in b.instructions)
        if not has_activation:
            return
        tables = get_activation_tables(nc.m.arch)
        if keep in tables and ours <= set(tables[keep]):
            tables = {name: (fns if name == keep else set(fns) - ours)
                      for name, fns in tables.items()}
        _bass_rust.insert_act_table_loads(nc, list(tables.items()))

    nc.insert_act_table_loads = patched_pass


def _host_prep(x, w_q, w_k, w_v, w_head, in_gamma, in_beta, w_proj, b_proj):
    """Build the 8 per-core input maps (all fp32 numpy)."""
    x = np.asarray(x, dtype=np.float32)
    w_q = np.asarray(w_q, dtype=np.float32)
    w_k = np.asarray(w_k, dtype=np.float32)
    w_v = np.asarray(w_v, dtype=np.float32)
    w_head = np.asarray(w_head, dtype=np.float32)

    wqT = np.ascontiguousarray(w_q.T) * np.float32(SCALE)
    wkT = np.ascontiguousarray(w_k.T)
    wpT_r = np.ascontiguousarray(
        np.asarray(w_proj, dtype=np.float32)
        .T.reshape(8, 64, C).transpose(1, 0, 2).reshape(64, 8 * C))

    in_maps = []
    p = np.arange(128)
    for core in range(NCORES):
        b = core // 2
        g0 = (core % 2) * GPC
        xc = np.ascontiguousarray(x[b].reshape(C, T))
        wvT = np.ascontiguousarray(w_v.T[:, g0 * HD:(g0 + GPC) * HD])
        al = np.empty((128, CC * GPC), dtype=np.float32)
        for cc in range(CC):
            for gi in range(GPC):
                al[:, cc * GPC + gi] = w_head[g0 + gi, cc * 2 + p // 64]
        in_maps.append({
            "x": xc, "wqT": wqT, "wkT": wkT, "wvT": wvT,
            "wpT": wpT_r, "alphas": al,
            "ones": np.ones((128, 128), dtype=np.float32),
        })
    return in_maps


def _host_finish(results, x, w_v, w_head, in_gamma, in_beta, w_proj, b_proj):
    in_gamma = np.asarray(in_gamma, dtype=np.float32)
    in_beta = np.asarray(in_beta, dtype=np.float32)
    w_proj = np.asarray(w_proj, dtype=np.float32)
    b_proj = np.asarray(b_proj, dtype=np.float32)
    w_v = np.asarray(w_v, dtype=np.float32)
    x = np.asarray(x, dtype=np.float32)

    # collapsed_wp[d, c] = sum_jh w_proj[c, jh*64+d]
    collapsed_wp = w_proj.reshape(C, 8, 64).sum(axis=1).T   # [64, C]
    out = np.empty((B, C, T), dtype=np.float32)
    for core in range(NCORES):
        b = core // 2
        g0 = (core % 2) * GPC
        dev = results[core]["out"]              # [512 i, 512 c]
        s2 = results[core]["s2"].sum(axis=1)    # [GPC]
        sv = w_v @ x[b].reshape(C, T).sum(axis=1)   # [C]
        for gi in range(GPC):
            g = g0 + gi
            var = s2[gi] / float(T * T) - MU * MU
            a = in_gamma[g] / np.sqrt(var + EPS)
            cs = in_beta[g] - a * MU
            bias2 = collapsed_wp.T @ sv[g * HD:(g + 1) * HD]   # [C]
            blk = dev[gi * 128:(gi + 1) * 128, :]              # [128 i, C]
            full = a * blk + (cs * bias2 + b_proj)[None, :]
            out[b, :, g * 128:(g + 1) * 128] = full.T
    return out.reshape(B, C, HH, WW)


_runner_cache = {}


def _make_runner(nc):
    """One-time jax.jit of the SPMD launch for ``nc`` (the per-call closure
    inside run_bass_kernel_spmd defeats jax's jit cache, costing a re-trace
    plus compile-cache lookup on every call)."""
    if id(nc) in _runner_cache:
        return _runner_cache[id(nc)]

    from concourse import bass2jax as b2j
    import concourse.mybir as _mybir

    b2j.install_neuronx_cc_hook()
    partition_name = (nc.partition_id_tensor.name
                      if nc.partition_id_tensor else None)
    in_names, out_names, out_avals, zero_shapes = [], [], [], []
    for alloc in nc.m.functions[0].allocations:
        if not isinstance(alloc, _mybir.MemoryLocationSet):
            continue
        name = alloc.memorylocations[0].name
        if alloc.kind == "ExternalInput":
            if name != partition_name:
                in_names.append(name)
        elif alloc.kind == "ExternalOutput":
            shape = tuple(alloc.tensor_shape)
            dtype = _mybir.dt.np(alloc.dtype)
            out_names.append(name)
            out_avals.append(jax.core.ShapedArray(shape, dtype))
            zero_shapes.append((shape, dtype))
    n_params = len(in_names)
    all_names = in_names + out_names + (
        [partition_name] if partition_name else [])
    donate = tuple(range(n_params, n_params + len(out_names)))

    def _body(*args):
        operands = list(args)
        if partition_name is not None:
            operands.append(b2j.partition_id_tensor())
        return tuple(b2j._bass_exec_p.bind(
            *operands, out_avals=tuple(out_avals), in_names=tuple(all_names),
            out_names=tuple(out_names), lowering_input_output_aliases=(),
            sim_require_finite=True, sim_require_nnan=True, nc=nc))

    from jax.experimental.shard_map import shard_map
    from jax.sharding import Mesh, PartitionSpec
    devices = jax.devices()[:NCORES]
    mesh = Mesh(np.asarray(devices), ("core",))
    nin = n_params + len(out_names)
    sharded = jax.jit(
        shard_map(_body, mesh=mesh, in_specs=(PartitionSpec("core"),) * nin,
                  out_specs=(PartitionSpec("core"),) * len(out_names),
                  check_rep=False),
        donate_argnums=donate, keep_unused=True)

    def run(in_maps):
        concat_in = [
            np.concatenate([np.asarray(m[name]) for m in in_maps], axis=0)
            for name in in_names]
        concat_zeros = [np.zeros((NCORES * s[0], *s[1:]), d)
                        for s, d in zero_shapes]
        outs = sharded(*concat_in, *concat_zeros)
        return [
            {name: np.asarray(outs[i]).reshape(NCORES, *out_avals[i].shape)[c]
             for i, name in enumerate(out_names)}
            for c in range(NCORES)]

    _runner_cache[id(nc)] = run
    return run


def _run(inputs, trace=False, reps=1):
    nc = build_program(reps)
    in_maps = _host_prep(**inputs)
    results = _make_runner(nc)(in_maps)
    out = _host_finish(results, inputs["x"], inputs["w_v"],
                       inputs["w_head"], inputs["in_gamma"],
                       inputs["in_beta"], inputs["w_proj"], inputs["b_proj"])
    return out, results


def kernel(**inputs):
    out, _ = _run(inputs, trace=False)
    return out
